# revision 17
# baseline (speedup 1.0000x reference)
"""Single-head causal attention on 8 TRN2 NeuronCores — v14 (2 loads, 4 chunk transposes).

Problem: x[B=8, T=2048, C=1024], Wq/Wk/Wv[C, H=64] (fp32)
  q = x@Wq; k = x@Wk; v = x@Wv
  wei = softmax(mask(q k^T * C^-0.5)); out = wei @ v       -> [B, T, H]

Sharding: data-parallel over batch, one batch element per core.

Per-core dataflow:
  - x loaded fp32 in 8x 1MB pieces on the gpsimd SWDGE queue.  SWDGE has
    its own completion-semaphore pool, so the loads never share lanes
    with the transposes (the HWDGE lane pool round-robins across queues
    and convoys the pipeline otherwise).  Cast fp32->bf16 per t-tile on
    DVE (2x mode) / ScalarE, xbar-transpose per t-tile on sync.
  - Projections packed so every S operand lands where it's needed with
    zero SBUF->SBUF copies:
      [Wk]    -> ka:  kT at partitions 0:64
      [Wq|Wv] -> qv:  qT at partitions 0:64, v at 64:128
    S^T block = ka_block.T @ qv[0:64]  (K=64, tile(0,0))
  - exp batched per block-pair [128,1024] across 2 PSUM banks.
  - causal mask = post-exp affine_select zeroing on bf16 pt (gpsimd).
  - PV accumulates [v|1]^T @ exp(S^T) -> row 64 gives sumexp for free;
    PE-transpose + reciprocal + scale for the final [T,H] output.
"""
import sys

sys.path.insert(0, "/opt/trn_rl_repo")

import numpy as np

import concourse.bass as bass
import concourse.mybir as mybir
import concourse.tile as tile
from concourse import bacc
from concourse.bass_utils import run_bass_kernel_spmd
from concourse.masks import make_identity

B, T, C, H = 8, 2048, 1024, 64
NTT = T // 128   # 16 t-tiles
NCT = C // 128   # 8  c-tiles
NCH = T // 512   # 4  t-chunks
SCALE = float(C) ** -0.5
VP = 80          # v_nat per-tile stride: 160B, 32B-aligned for xbar transpose

F32 = mybir.dt.float32
BF16 = mybir.dt.bfloat16


def build_nc(reps=1):
    nc = bacc.Bacc("TRN2", target_bir_lowering=False, debug=False,
                   dynamic_dma_scratch_size=49152)
    xD = nc.dram_tensor("x", [T, C], F32, kind="ExternalInput").ap()
    wqD = nc.dram_tensor("Wq", [128, NCT, H], F32, kind="ExternalInput").ap()
    wkD = nc.dram_tensor("Wk", [128, NCT, H], F32, kind="ExternalInput").ap()
    wvD = nc.dram_tensor("Wv", [128, NCT, H], F32, kind="ExternalInput").ap()
    outD = nc.dram_tensor("out", [T, H], F32, kind="ExternalOutput").ap()

    AF = mybir.ActivationFunctionType

    with tile.TileContext(nc) as tc:
        with (
            tc.tile_pool(name="const", bufs=1) as cpool,
            tc.tile_pool(name="xnat", bufs=1) as xnpool,
            tc.tile_pool(name="xt", bufs=1) as xtpool,
            tc.tile_pool(name="qk", bufs=1) as qkpool,
            tc.tile_pool(name="pt", bufs=4) as ptpool,
            tc.tile_pool(name="osb", bufs=3) as opool,
            tc.tile_pool(name="fin", bufs=2) as fpool,
        ):
            # ---- constants ----
            ident = cpool.tile([128, 128], F32)
            make_identity(nc, ident[:])
            # W loads ride the scalar HWDGE ring (front, one-shot); the
            # SWDGE ring stays exclusively for the 8 x-piece loads.
            wqf = cpool.tile([128, NCT, H], F32)
            wvf = cpool.tile([128, NCT, H], F32)
            wkf = cpool.tile([128, NCT, H], F32)
            nc.scalar.dma_start(wqf[:], wqD)
            nc.scalar.dma_start(wvf[:], wvD)
            nc.scalar.dma_start(wkf[:], wkD)
            wqv = cpool.tile([128, NCT, 128], BF16)
            wk = cpool.tile([128, NCT, H], BF16)
            nc.vector.tensor_copy(wqv[:, :, 0:H], wqf[:])
            nc.vector.tensor_copy(wqv[:, :, H:128], wvf[:])
            nc.vector.tensor_copy(wk[:], wkf[:])

            scrap = cpool.tile([128, 1], F32)

            for rep in range(reps):
                emit_body(nc, tc, xD, outD,
                          (wqv, wk, ident, scrap),
                          (xnpool, xtpool, qkpool, ptpool, opool, fpool))

    nc.compile()
    return nc


def emit_body(nc, tc, xD, outD, consts, pools):
    AF = mybir.ActivationFunctionType
    ALU = mybir.AluOpType
    wqv, wk, ident, scrap = consts
    xnpool, xtpool, qkpool, ptpool, opool, fpool = pools

    x_nat = xnpool.tile([128, NTT, C], F32, tag="xnat")
    x_natb = xnpool.tile([128, NTT, C], BF16, tag="xnatb")
    xt = xtpool.tile([128, NTT, NCT, 128], BF16, tag="xt")
    xR = xD.rearrange("(g p) c -> p g c", p=128)

    qv = qkpool.tile([128, T], BF16, tag="qv")    # rows 0:64 qT, 64:128 v
    ka = qkpool.tile([64, T], BF16, tag="ka")     # kT at partitions 0:64
    v_nat = qkpool.tile([128, NTT, VP], BF16, tag="vnat")
    nc.gpsimd.memset(v_nat[:, :, H:H + 1], 1.0)
    o_out = fpool.tile([128, NTT, H], F32, tag="oout")
    outR = outD.rearrange("(g p) h -> p g h", p=128)

    PIECES = ((0, 8), (8, 16))   # t-tile ranges per load piece

    def castc(ci):
        # cast + transpose one 512-t chunk: t-tiles share the same 128
        # partitions, so one xbar transpose of [128, 4096] lands each
        # 128-col group in its own (tile, c_grp) slot of xt.  With only
        # 2 loads queued first, the scheduler's op(k+2)-waits-
        # transpose(k) window only ever chains transposes to transposes,
        # which serialize on the sync queue anyway.
        nc.vector.tensor_copy(x_natb[:, 4 * ci:4 * ci + 4, :],
                              x_nat[:, 4 * ci:4 * ci + 4, :])
        nc.sync.dma_start(
            xt[:, 4 * ci:4 * ci + 4, :, :], x_natb[:, 4 * ci:4 * ci + 4, :],
            transpose=True,
        )

    with (
        tc.tile_pool(name="qkps", bufs=1, space="PSUM") as qkps,
        tc.tile_pool(name="aux", bufs=1, space="PSUM") as aux,
        tc.tile_pool(name="ops", bufs=2, space="PSUM") as ops,
        tc.tile_pool(name="stps", bufs=2, space="PSUM") as stps,
    ):
        vps = fps = aux

        def emit_warm(n):
            # PE warm-up gated on the weight cast (ready ~3us); keeps HAM
            # warm through the load lead-in until QKV(0).
            warm = qkps.tile([128, 512], F32, tag="psqk")
            for _ in range(n):
                nc.tensor.matmul(
                    warm[:], wqv[:, 0, :], wqv[:, 0:4, :].opt(),
                    start=True, stop=True,
                )

        def emit_qkv(ci):
            sl = slice(ci * 512, (ci + 1) * 512)
            ps_qv = vps.tile([128, 512], F32, tag="aux")
            for k in range(NCT):
                nc.tensor.matmul(
                    ps_qv[:], wqv[:, k, :], xt[:, ci * 4:(ci + 1) * 4, k, :],
                    start=(k == 0), stop=(k == NCT - 1),
                )
            nc.vector.tensor_copy(qv[:, sl], ps_qv[:])
            nc.sync.dma_start(
                v_nat[:, ci * 4:(ci + 1) * 4, 0:H], qv[64:128, sl],
                transpose=True,
            )
            ps_k_t = qkps.tile([128, 512], F32, tag="psqk")
            ps_k = ps_k_t[0:64, :]
            for k in range(NCT):
                nc.tensor.matmul(
                    ps_k[:], wk[:, k, :], xt[:, ci * 4:(ci + 1) * 4, k, :],
                    start=(k == 0), stop=(k == NCT - 1),
                )
            nc.vector.tensor_copy(ka[:, sl], ps_k[:])

        out_pcs = {}

        def emit_attn_core(ci):
            out_pc = ops.tile([H + 1, 512], F32, tag="outc")
            out_pcs[ci] = out_pc
            npair = 2 * ci + 2
            nsb = 4 * ci + 4
            cl, cr = ci * 512, (ci + 1) * 512
            pending = []
            for p in range(npair):
                sbe, sbo = 2 * p, 2 * p + 1
                re, ro = sbe - 4 * ci, sbo - 4 * ci
                t0e, t0o = max(re, 0) * 128, max(ro, 0) * 128
                st = stps.tile([128, 1024], F32, tag="st")
                nc.tensor.matmul(
                    st[:, t0e:512],
                    ka[0:64, sbe * 128:(sbe + 1) * 128],
                    qv[0:64, cl + t0e:cr],
                    start=True, stop=True,
                )
                nc.tensor.matmul(
                    st[:, 512 + t0o:1024],
                    ka[0:64, sbo * 128:(sbo + 1) * 128],
                    qv[0:64, cl + t0o:cr],
                    start=True, stop=True,
                )
                pt = ptpool.tile([128, 1024], BF16, tag="pt")
                if re < 0:  # fully off-diagonal pair: one batched exp
                    nc.scalar.activation(
                        pt[:, 0:1024], st[:, 0:1024], AF.Exp, scale=SCALE)
                else:
                    nc.scalar.activation(
                        pt[:, t0e:512], st[:, t0e:512], AF.Exp, scale=SCALE)
                    nc.scalar.activation(
                        pt[:, 512 + t0o:1024], st[:, 512 + t0o:1024],
                        AF.Exp, scale=SCALE)
                    # zero upper triangle of the diagonal 128-blocks
                    nc.gpsimd.affine_select(
                        out=pt[:, t0e:t0e + 128], in_=pt[:, t0e:t0e + 128],
                        compare_op=ALU.is_ge, fill=0.0,
                        base=0, pattern=[[1, 128]], channel_multiplier=-1,
                    )
                    nc.gpsimd.affine_select(
                        out=pt[:, 512 + t0o:512 + t0o + 128],
                        in_=pt[:, 512 + t0o:512 + t0o + 128],
                        compare_op=ALU.is_ge, fill=0.0,
                        base=0, pattern=[[1, 128]], channel_multiplier=-1,
                    )
                if pending:
                    for args, kw in pending:
                        nc.tensor.matmul(*args, **kw)
                pending = [
                    ((out_pc[:, t0e:512], v_nat[:, sbe, 0:H + 1],
                      pt[:, t0e:512]),
                     dict(start=(sbe == 0), stop=False)),
                    ((out_pc[:, t0o:512], v_nat[:, sbo, 0:H + 1],
                      pt[:, 512 + t0o:1024]),
                     dict(start=False, stop=(sbo == nsb - 1))),
                ]
            for args, kw in pending:
                nc.tensor.matmul(*args, **kw)

        def emit_attn_out(ci):
            out_pc = out_pcs[ci]
            o_c = opool.tile([H + 1, 512], F32, tag="osb")
            nc.vector.tensor_copy(o_c[:], out_pc[:])
            fin_t = fps.tile([128, 4, 128], F32, tag="aux")
            fin4 = fin_t[:, :, 0:H + 1]
            for rr in range(4):
                nc.tensor.transpose(
                    fin4[:, rr, :],
                    o_c[:, rr * 128:(rr + 1) * 128],
                    ident[0:H + 1, 0:H + 1],
                )
            rcp = fpool.tile([128, 4, 1], F32, tag="rcp")
            nc.vector.reciprocal(rcp[:], fin4[:, :, H:H + 1])
            nc.vector.tensor_tensor(
                o_out[:, ci * 4:(ci + 1) * 4, :], fin4[:, :, 0:H],
                rcp[:].to_broadcast([128, 4, H]), op=ALU.mult,
            )
            nc.scalar.dma_start(
                outR[:, ci * 4:(ci + 1) * 4, :],
                o_out[:, ci * 4:(ci + 1) * 4, :],
            )

        # ---- three x loads queued up-front on the scalar HWDGE ring
        # (HWDGE streams ~360 GB/s; SWDGE measured only ~175 GB/s here).
        for lo, hi in PIECES:
            nc.scalar.dma_start(x_nat[:, lo:hi, :], xR[:, lo:hi, :])
        # table preload: first Exp triggers ACT_TABLE_LOAD early (after
        # the load issues so it doesn't head-block them in the ACT FIFO)
        nc.scalar.activation(scrap[:], ident[:, 0:1], AF.Exp)

        emit_warm(40)
        castc(0)
        castc(1)
        emit_qkv(0)
        castc(2)
        emit_attn_core(0)
        emit_qkv(1)
        castc(3)
        emit_attn_core(1)
        emit_attn_out(0)
        emit_qkv(2)
        emit_attn_core(2)
        emit_attn_out(1)
        emit_qkv(3)
        emit_attn_out(2)
        emit_attn_core(3)
        emit_attn_out(3)


_NC = None


def kernel(x, Wq, Wk, Wv):
    global _NC
    if _NC is None:
        _NC = build_nc()
    def wperm(W):
        return np.ascontiguousarray(
            np.asarray(W, dtype=np.float32).reshape(NCT, 128, H)
            .transpose(1, 0, 2))

    WqP, WkP, WvP = wperm(Wq), wperm(Wk), wperm(Wv)
    in_maps = [
        {
            "x": np.ascontiguousarray(x[b], dtype=np.float32),
            "Wq": WqP, "Wk": WkP, "Wv": WvP,
        }
        for b in range(B)
    ]
    res = run_bass_kernel_spmd(_NC, in_maps, core_ids=list(range(B)))
    return np.stack([res.results[b]["out"] for b in range(B)], axis=0)


# revision 18
# speedup vs baseline: 1.1105x; 1.1105x over previous
"""Single-head causal attention on 8 TRN2 NeuronCores — v12 (half-granularity load/transpose).

Problem: x[B=8, T=2048, C=1024], Wq/Wk/Wv[C, H=64] (fp32)
  q = x@Wq; k = x@Wk; v = x@Wv
  wei = softmax(mask(q k^T * C^-0.5)); out = wei @ v       -> [B, T, H]

Sharding: data-parallel over batch, one batch element per core.

Per-core dataflow:
  - x loaded fp32 in 8x 1MB pieces on the gpsimd SWDGE queue.  SWDGE has
    its own completion-semaphore pool, so the loads never share lanes
    with the transposes (the HWDGE lane pool round-robins across queues
    and convoys the pipeline otherwise).  Cast fp32->bf16 per t-tile on
    DVE (2x mode) / ScalarE, xbar-transpose per t-tile on sync.
  - Projections packed so every S operand lands where it's needed with
    zero SBUF->SBUF copies:
      [Wk]    -> ka:  kT at partitions 0:64
      [Wq|Wv] -> qv:  qT at partitions 0:64, v at 64:128
    S^T block = ka_block.T @ qv[0:64]  (K=64, tile(0,0))
  - exp batched per block-pair [128,1024] across 2 PSUM banks.
  - causal mask = post-exp affine_select zeroing on bf16 pt (gpsimd).
  - PV accumulates [v|1]^T @ exp(S^T) -> row 64 gives sumexp for free;
    PE-transpose + reciprocal + scale for the final [T,H] output.
"""
import sys

sys.path.insert(0, "/opt/trn_rl_repo")

import numpy as np

import concourse.bass as bass
import concourse.mybir as mybir
import concourse.tile as tile
from concourse import bacc
from concourse.bass_utils import run_bass_kernel_spmd
from concourse.masks import make_identity

B, T, C, H = 8, 2048, 1024, 64
NTT = T // 128   # 16 t-tiles
NCT = C // 128   # 8  c-tiles
NCH = T // 512   # 4  t-chunks
SCALE = float(C) ** -0.5
VP = 80          # v_nat per-tile stride: 160B, 32B-aligned for xbar transpose

F32 = mybir.dt.float32
BF16 = mybir.dt.bfloat16


def build_nc(reps=1):
    nc = bacc.Bacc("TRN2", target_bir_lowering=False, debug=False,
                   dynamic_dma_scratch_size=49152)
    xD = nc.dram_tensor("x", [T, C], F32, kind="ExternalInput").ap()
    wqD = nc.dram_tensor("Wq", [128, NCT, H], F32, kind="ExternalInput").ap()
    wkD = nc.dram_tensor("Wk", [128, NCT, H], F32, kind="ExternalInput").ap()
    wvD = nc.dram_tensor("Wv", [128, NCT, H], F32, kind="ExternalInput").ap()
    outD = nc.dram_tensor("out", [T, H], F32, kind="ExternalOutput").ap()

    AF = mybir.ActivationFunctionType

    with tile.TileContext(nc) as tc:
        with (
            tc.tile_pool(name="const", bufs=1) as cpool,
            tc.tile_pool(name="xnat", bufs=1) as xnpool,
            tc.tile_pool(name="xt", bufs=1) as xtpool,
            tc.tile_pool(name="qk", bufs=1) as qkpool,
            tc.tile_pool(name="pt", bufs=4) as ptpool,
            tc.tile_pool(name="osb", bufs=3) as opool,
            tc.tile_pool(name="fin", bufs=2) as fpool,
        ):
            # ---- constants ----
            ident = cpool.tile([128, 128], F32)
            make_identity(nc, ident[:])
            # W loads ride the scalar HWDGE ring (front, one-shot); the
            # SWDGE ring stays exclusively for the 8 x-piece loads.
            wqf = cpool.tile([128, NCT, H], F32)
            wvf = cpool.tile([128, NCT, H], F32)
            wkf = cpool.tile([128, NCT, H], F32)
            nc.scalar.dma_start(wqf[:], wqD)
            nc.scalar.dma_start(wvf[:], wvD)
            nc.scalar.dma_start(wkf[:], wkD)
            wqv = cpool.tile([128, NCT, 128], BF16)
            wk = cpool.tile([128, NCT, H], BF16)
            nc.vector.tensor_copy(wqv[:, :, 0:H], wqf[:])
            nc.vector.tensor_copy(wqv[:, :, H:128], wvf[:])
            nc.vector.tensor_copy(wk[:], wkf[:])

            scrap = cpool.tile([128, 1], F32)

            for rep in range(reps):
                emit_body(nc, tc, xD, outD,
                          (wqv, wk, ident, scrap),
                          (xnpool, xtpool, qkpool, ptpool, opool, fpool))

    nc.compile()
    return nc


def emit_body(nc, tc, xD, outD, consts, pools):
    AF = mybir.ActivationFunctionType
    ALU = mybir.AluOpType
    wqv, wk, ident, scrap = consts
    xnpool, xtpool, qkpool, ptpool, opool, fpool = pools

    x_nat = xnpool.tile([128, NTT, C], F32, tag="xnat")
    x_natb = xnpool.tile([128, NTT, C], BF16, tag="xnatb")
    xt = xtpool.tile([128, NTT, NCT, 128], BF16, tag="xt")
    xR = xD.rearrange("(g p) c -> p g c", p=128)

    qv = qkpool.tile([128, T], BF16, tag="qv")    # rows 0:64 qT, 64:128 v
    ka = qkpool.tile([64, T], BF16, tag="ka")     # kT at partitions 0:64
    v_nat = qkpool.tile([128, NTT, VP], BF16, tag="vnat")
    nc.gpsimd.memset(v_nat[:, :, H:H + 1], 1.0)
    o_out = fpool.tile([128, NTT, H], F32, tag="oout")
    outR = outD.rearrange("(g p) h -> p g h", p=128)

    PIECES = ((0, 8), (8, 16))   # t-tile ranges per load piece

    def casth(h):
        # cast + transpose HALF of x (8 t-tiles) in one op each: t-tiles
        # share the same 128 partitions, so one xbar transpose of
        # [128, 8192] lands each 128-col group in its own (tile, c_grp)
        # slot of xt.  The scheduler serializes dynamic DMA op k+2 behind
        # transpose k regardless of queue, so with 2 loads + 2 transposes
        # the window never binds at all.
        nc.vector.tensor_copy(x_natb[:, 8 * h:8 * h + 8, :],
                              x_nat[:, 8 * h:8 * h + 8, :])
        nc.sync.dma_start(
            xt[:, 8 * h:8 * h + 8, :, :], x_natb[:, 8 * h:8 * h + 8, :],
            transpose=True,
        )

    with (
        tc.tile_pool(name="qkps", bufs=1, space="PSUM") as qkps,
        tc.tile_pool(name="aux", bufs=1, space="PSUM") as aux,
        tc.tile_pool(name="ops", bufs=2, space="PSUM") as ops,
        tc.tile_pool(name="stps", bufs=2, space="PSUM") as stps,
    ):
        vps = fps = aux

        def emit_warm(n):
            # PE warm-up gated on the weight cast (ready ~3us); keeps HAM
            # warm through the load lead-in until QKV(0).
            warm = qkps.tile([128, 512], F32, tag="psqk")
            for _ in range(n):
                nc.tensor.matmul(
                    warm[:], wqv[:, 0, :], wqv[:, 0:4, :].opt(),
                    start=True, stop=True,
                )

        def emit_qkv(ci):
            sl = slice(ci * 512, (ci + 1) * 512)
            ps_qv = vps.tile([128, 512], F32, tag="aux")
            for k in range(NCT):
                nc.tensor.matmul(
                    ps_qv[:], wqv[:, k, :], xt[:, ci * 4:(ci + 1) * 4, k, :],
                    start=(k == 0), stop=(k == NCT - 1),
                )
            nc.vector.tensor_copy(qv[:, sl], ps_qv[:])
            nc.sync.dma_start(
                v_nat[:, ci * 4:(ci + 1) * 4, 0:H], qv[64:128, sl],
                transpose=True,
            )
            ps_k_t = qkps.tile([128, 512], F32, tag="psqk")
            ps_k = ps_k_t[0:64, :]
            for k in range(NCT):
                nc.tensor.matmul(
                    ps_k[:], wk[:, k, :], xt[:, ci * 4:(ci + 1) * 4, k, :],
                    start=(k == 0), stop=(k == NCT - 1),
                )
            nc.vector.tensor_copy(ka[:, sl], ps_k[:])

        out_pcs = {}

        def emit_attn_core(ci):
            out_pc = ops.tile([H + 1, 512], F32, tag="outc")
            out_pcs[ci] = out_pc
            npair = 2 * ci + 2
            nsb = 4 * ci + 4
            cl, cr = ci * 512, (ci + 1) * 512
            pending = []
            for p in range(npair):
                sbe, sbo = 2 * p, 2 * p + 1
                re, ro = sbe - 4 * ci, sbo - 4 * ci
                t0e, t0o = max(re, 0) * 128, max(ro, 0) * 128
                st = stps.tile([128, 1024], F32, tag="st")
                nc.tensor.matmul(
                    st[:, t0e:512],
                    ka[0:64, sbe * 128:(sbe + 1) * 128],
                    qv[0:64, cl + t0e:cr],
                    start=True, stop=True,
                )
                nc.tensor.matmul(
                    st[:, 512 + t0o:1024],
                    ka[0:64, sbo * 128:(sbo + 1) * 128],
                    qv[0:64, cl + t0o:cr],
                    start=True, stop=True,
                )
                pt = ptpool.tile([128, 1024], BF16, tag="pt")
                if re < 0:  # fully off-diagonal pair: one batched exp
                    nc.scalar.activation(
                        pt[:, 0:1024], st[:, 0:1024], AF.Exp, scale=SCALE)
                else:
                    nc.scalar.activation(
                        pt[:, t0e:512], st[:, t0e:512], AF.Exp, scale=SCALE)
                    nc.scalar.activation(
                        pt[:, 512 + t0o:1024], st[:, 512 + t0o:1024],
                        AF.Exp, scale=SCALE)
                    # zero upper triangle of the diagonal 128-blocks
                    nc.gpsimd.affine_select(
                        out=pt[:, t0e:t0e + 128], in_=pt[:, t0e:t0e + 128],
                        compare_op=ALU.is_ge, fill=0.0,
                        base=0, pattern=[[1, 128]], channel_multiplier=-1,
                    )
                    nc.gpsimd.affine_select(
                        out=pt[:, 512 + t0o:512 + t0o + 128],
                        in_=pt[:, 512 + t0o:512 + t0o + 128],
                        compare_op=ALU.is_ge, fill=0.0,
                        base=0, pattern=[[1, 128]], channel_multiplier=-1,
                    )
                if pending:
                    for args, kw in pending:
                        nc.tensor.matmul(*args, **kw)
                pending = [
                    ((out_pc[:, t0e:512], v_nat[:, sbe, 0:H + 1],
                      pt[:, t0e:512]),
                     dict(start=(sbe == 0), stop=False)),
                    ((out_pc[:, t0o:512], v_nat[:, sbo, 0:H + 1],
                      pt[:, 512 + t0o:1024]),
                     dict(start=False, stop=(sbo == nsb - 1))),
                ]
            for args, kw in pending:
                nc.tensor.matmul(*args, **kw)

        def emit_attn_out(ci):
            out_pc = out_pcs[ci]
            o_c = opool.tile([H + 1, 512], F32, tag="osb")
            nc.vector.tensor_copy(o_c[:], out_pc[:])
            fin_t = fps.tile([128, 4, 128], F32, tag="aux")
            fin4 = fin_t[:, :, 0:H + 1]
            for rr in range(4):
                nc.tensor.transpose(
                    fin4[:, rr, :],
                    o_c[:, rr * 128:(rr + 1) * 128],
                    ident[0:H + 1, 0:H + 1],
                )
            rcp = fpool.tile([128, 4, 1], F32, tag="rcp")
            nc.vector.reciprocal(rcp[:], fin4[:, :, H:H + 1])
            nc.vector.tensor_tensor(
                o_out[:, ci * 4:(ci + 1) * 4, :], fin4[:, :, 0:H],
                rcp[:].to_broadcast([128, 4, H]), op=ALU.mult,
            )
            nc.scalar.dma_start(
                outR[:, ci * 4:(ci + 1) * 4, :],
                o_out[:, ci * 4:(ci + 1) * 4, :],
            )

        # ---- three x loads queued up-front on the scalar HWDGE ring
        # (HWDGE streams ~360 GB/s; SWDGE measured only ~175 GB/s here).
        for lo, hi in PIECES:
            nc.scalar.dma_start(x_nat[:, lo:hi, :], xR[:, lo:hi, :])
        # table preload: first Exp triggers ACT_TABLE_LOAD early (after
        # the load issues so it doesn't head-block them in the ACT FIFO)
        nc.scalar.activation(scrap[:], ident[:, 0:1], AF.Exp)

        emit_warm(34)
        casth(0)
        casth(1)
        emit_qkv(0)
        emit_attn_core(0)
        emit_qkv(1)
        emit_attn_core(1)
        emit_attn_out(0)
        emit_qkv(2)
        emit_attn_core(2)
        emit_attn_out(1)
        emit_qkv(3)
        emit_attn_out(2)
        emit_attn_core(3)
        emit_attn_out(3)


_NC = None


def kernel(x, Wq, Wk, Wv):
    global _NC
    if _NC is None:
        _NC = build_nc()
    def wperm(W):
        return np.ascontiguousarray(
            np.asarray(W, dtype=np.float32).reshape(NCT, 128, H)
            .transpose(1, 0, 2))

    WqP, WkP, WvP = wperm(Wq), wperm(Wk), wperm(Wv)
    in_maps = [
        {
            "x": np.ascontiguousarray(x[b], dtype=np.float32),
            "Wq": WqP, "Wk": WkP, "Wv": WvP,
        }
        for b in range(B)
    ]
    res = run_bass_kernel_spmd(_NC, in_maps, core_ids=list(range(B)))
    return np.stack([res.results[b]["out"] for b in range(B)], axis=0)
